# revision 13
# baseline (speedup 1.0000x reference)
"""Trainium2 Bass kernel for nn_ConvBN2d_if (ConvBN2d + integrate-and-fire SNN layer).

Reference semantics (N=32, T=10, Cin=Cout=128, H=W=32, 3x3 conv, pad 1):
  ratio  = bn_gamma / sqrt(bn_var)
  w_fold = conv_w * ratio[:,None,None,None]
  b_fold = (conv_b - bn_mean) * ratio + bn_beta
  pots[n,t] = conv(st[n,t], w_fold)                 # SNN path
  mem = b_fold; for t: mem += pots[t]; spike = mem > 1; mem -= spike
  output_features_st = spikes                       # [N,T,Cout,32,32]
  output_features_sc = relu(BN(conv(sc)))-based straight-through whose
                       forward value is exactly spike_count (up to one
                       fp32 rounding), so we return the spike count.

Strategy: data-parallel over N across 8 cores (4 samples/core). The 3x3
conv is 9 shifted matmuls on a zero-padded [128, 34*34] spike image per
(n,t), accumulated straight onto the membrane state held in PSUM.
Weights are scaled by 128 and split into fp16 hi+lo (2 matmul passes);
spike inputs are fed as 0 / (1/128) in fp16, so hi*x + lo*x accumulated
in fp32 PSUM reproduces fp32-conv numerics to ~1e-7 relative. Spikes
out as fp16 0/1 (exact); counts accumulate in fp16 (integers <= 10,
exact) and are upcast on the host.
"""
import os
import time
import numpy as np
from contextlib import ExitStack

import concourse.tile as tile
import concourse.mybir as mybir
from concourse import bacc, bass_utils

N_TOTAL, T, CIN, COUT, H, W = 32, 10, 128, 128, 32, 32
NCORES = 8
NPC = N_TOTAL // NCORES          # samples per core
HP = WP = 34                     # padded image
HW = H * W                       # 1024
NCH = 512                        # matmul free-dim chunk (1 PSUM bank)
SCALE = np.float32(128.0)        # weight scale; x carries 1/128

_cache = {}


def _build_program(reps=1):
    """reps>1 repeats the whole per-core body (for benchmarking): same
    xpad input, distinct output regions per rep."""
    nsamp = NPC * reps
    nc = bacc.Bacc("TRN2", target_bir_lowering=False, debug=False)
    f16, f32 = mybir.dt.float16, mybir.dt.float32

    xpad_d = nc.dram_tensor("xpad", [NPC, T, CIN, HP * WP], f16, kind="ExternalInput").ap()
    whi_d = nc.dram_tensor("whi", [9, CIN, COUT], f16, kind="ExternalInput").ap()
    wlo_d = nc.dram_tensor("wlo", [9, CIN, COUT], f16, kind="ExternalInput").ap()
    bias_d = nc.dram_tensor("bias", [1, 2 * COUT], f16, kind="ExternalInput").ap()
    spk_d = nc.dram_tensor("spk", [nsamp, T, COUT, HW], f16, kind="ExternalOutput").ap()
    cnt_d = nc.dram_tensor("cnt", [nsamp, COUT, HW], f16, kind="ExternalOutput").ap()

    with tile.TileContext(nc) as tc, ExitStack() as ctx:
        const = ctx.enter_context(tc.tile_pool(name="const", bufs=1))
        xpool = ctx.enter_context(tc.tile_pool(name="xpool", bufs=4))
        spool = ctx.enter_context(tc.tile_pool(name="spool", bufs=6))
        cpool = ctx.enter_context(tc.tile_pool(name="cpool", bufs=1))
        mpool = ctx.enter_context(tc.tile_pool(name="mpool", bufs=1, space="PSUM"))

        bias_t = const.tile([1, 2 * COUT], f16)
        nc.sync.dma_start(bias_t[:], bias_d[:])
        ones_t = const.tile([1, NCH], f16)
        nc.gpsimd.memset(ones_t[:], 1.0 / 128.0)
        whi_t = const.tile([CIN, 9 * COUT], f16)
        nc.sync.dma_start(whi_t.rearrange("p (k c) -> p k c", k=9),
                          whi_d.rearrange("k p c -> p k c"))
        wlo_t = const.tile([CIN, 9 * COUT], f16)
        nc.sync.dma_start(wlo_t.rearrange("p (k c) -> p k c", k=9),
                          wlo_d.rearrange("k p c -> p k c"))

        for rep in range(reps):
            # all NPC membranes resident: 4 tiles x 2 banks = all 8 PSUM
            # banks. t-outer / n-inner interleave means each sample's DVE
            # threshold work hides behind the other samples' matmuls.
            mems = [mpool.tile([COUT, 2 * NCH], f32, tag=f"mem{n}",
                               name=f"mem{n}_{rep}") for n in range(NPC)]
            cnts = [cpool.tile([COUT, HW], f16, tag=f"cnt{n}",
                               name=f"cnt{n}_{rep}") for n in range(NPC)]

            # membrane init: mem = b_fold (hi+lo), via K=1 matmuls so the
            # PSUM has_written bits are set by the PE (start=True).
            for n in range(NPC):
                nc.gpsimd.memset(cnts[n][:], 0.0)
                for c in range(2):
                    m = mems[n][:, NCH * c:NCH * (c + 1)]
                    nc.tensor.matmul(m, bias_t[0:1, 0:COUT], ones_t[:],
                                     start=True, stop=False)
                    nc.tensor.matmul(m, bias_t[0:1, COUT:2 * COUT], ones_t[:],
                                     start=False, stop=False)

            for t in range(T):
                last_t = t == T - 1
                for n in range(NPC):
                    mem = mems[n]
                    xp_t = xpool.tile([CIN, HP * WP], f16)
                    nc.sync.dma_start(xp_t[:], xpad_d[n, t])
                    xv = xp_t.rearrange("p (h w) -> p h w", w=WP)

                    for p_i, w_t in enumerate((whi_t, wlo_t)):
                        for k in range(9):
                            ky, kx = divmod(k, 3)
                            lhsT = w_t[:, k * COUT:(k + 1) * COUT]
                            for c in range(2):
                                nc.tensor.matmul(
                                    mem[:, NCH * c:NCH * (c + 1)], lhsT,
                                    xv[:, 16 * c + ky:16 * c + ky + 16, kx:kx + 32],
                                    start=False,
                                    stop=(last_t and p_i == 1 and k == 8))

                    spk_t = spool.tile([COUT, HW], f16)
                    nc.vector.tensor_scalar(spk_t[:], mem[:], 1.0, None,
                                            mybir.AluOpType.is_gt)
                    if not last_t:
                        nc.vector.tensor_tensor(mem[:], mem[:], spk_t[:],
                                                mybir.AluOpType.subtract)
                        nc.gpsimd.tensor_tensor(cnts[n][:], cnts[n][:], spk_t[:],
                                                mybir.AluOpType.add)
                    nc.sync.dma_start(spk_d[rep * NPC + n, t], spk_t[:])

            for n in range(NPC):
                nc.sync.dma_start(cnt_d[rep * NPC + n], cnts[n][:])

    nc.compile()
    return nc


def _get_program():
    if "nc" not in _cache:
        _cache["nc"] = _build_program()
    return _cache["nc"]


def kernel(input_feature_st, input_features_sc, conv_w, conv_b,
           bn_gamma, bn_beta, bn_mean, bn_var):
    st = np.asarray(input_feature_st, dtype=np.float32)
    conv_w = np.asarray(conv_w, dtype=np.float32)
    conv_b = np.asarray(conv_b, dtype=np.float32)
    bn_gamma = np.asarray(bn_gamma, dtype=np.float32)
    bn_beta = np.asarray(bn_beta, dtype=np.float32)
    bn_mean = np.asarray(bn_mean, dtype=np.float32)
    bn_var = np.asarray(bn_var, dtype=np.float32)

    nc = _get_program()

    ratio = bn_gamma / np.sqrt(bn_var)
    w_fold = conv_w * ratio[:, None, None, None]          # [co, ci, kh, kw]
    b_fold = (conv_b - bn_mean) * ratio + bn_beta         # [co]

    # [9, ci, co] scaled weight splits
    w9 = np.ascontiguousarray(w_fold.transpose(2, 3, 1, 0).reshape(9, CIN, COUT)) * SCALE
    whi = w9.astype(np.float16)
    wlo = (w9 - whi.astype(np.float32)).astype(np.float16)
    bs = b_fold * SCALE
    bhi = bs.astype(np.float16)
    blo = (bs - bhi.astype(np.float32)).astype(np.float16)
    bias = np.concatenate([bhi, blo]).reshape(1, 2 * COUT)

    # host-side zero-pad to 34x34, value 1/128 where spiking (exact in fp16:
    # bit pattern 0x2000). Integer path is ~2x faster than a float cast.
    xpad_u = np.zeros((N_TOTAL, T, CIN, HP, WP), np.uint16)
    xpad_u[:, :, :, 1:H + 1, 1:W + 1] = st.astype(np.uint8) * np.uint16(0x2000)
    xpad = xpad_u.view(np.float16).reshape(N_TOTAL, T, CIN, HP * WP)

    in_maps = [{"xpad": xpad[c * NPC:(c + 1) * NPC],
                "whi": whi, "wlo": wlo, "bias": bias} for c in range(NCORES)]
    # NTFF tracing is not available under this axon build; force it off so a
    # stray BASS_TRACE env var can't break execution.
    os.environ.setdefault("BASS_NEVER_TRACE", "1")
    res = None
    for attempt in range(3):
        try:
            res = bass_utils.run_bass_kernel_spmd(nc, in_maps, list(range(NCORES)))
            break
        except Exception:
            # transient NRT/device hiccups recover on retry
            if attempt == 2:
                raise
            time.sleep(2.0)
    _cache["last_result"] = res

    out_st = np.empty((N_TOTAL, T, COUT, H, W), np.float32)
    out_sc = np.empty((N_TOTAL, COUT, H, W), np.float32)
    for c in range(NCORES):
        r = res.results[c]
        out_st[c * NPC:(c + 1) * NPC] = (
            r["spk"].reshape(NPC, T, COUT, H, W).astype(np.float32))
        out_sc[c * NPC:(c + 1) * NPC] = (
            r["cnt"].reshape(NPC, COUT, H, W).astype(np.float32))
    # device accumulates t=0..8 only; t=9's spikes are added here (exact ints)
    out_sc += out_st[:, T - 1]
    return out_st, out_sc


# revision 14
# speedup vs baseline: 1.0109x; 1.0109x over previous
"""Trainium2 Bass kernel for nn_ConvBN2d_if (ConvBN2d + integrate-and-fire SNN layer).

Reference semantics (N=32, T=10, Cin=Cout=128, H=W=32, 3x3 conv, pad 1):
  ratio  = bn_gamma / sqrt(bn_var)
  w_fold = conv_w * ratio[:,None,None,None]
  b_fold = (conv_b - bn_mean) * ratio + bn_beta
  pots[n,t] = conv(st[n,t], w_fold)                 # SNN path
  mem = b_fold; for t: mem += pots[t]; spike = mem > 1; mem -= spike
  output_features_st = spikes                       # [N,T,Cout,32,32]
  output_features_sc = relu(BN(conv(sc)))-based straight-through whose
                       forward value is exactly spike_count (up to one
                       fp32 rounding), so we return the spike count.

Strategy: data-parallel over N across 8 cores (4 samples/core). The 3x3
conv is 9 shifted matmuls on a zero-padded [128, 34*34] spike image per
(n,t), accumulated straight onto the membrane state held in PSUM.
Weights are scaled by 128 and split into fp16 hi+lo (2 matmul passes);
spike inputs are fed as 0 / (1/128) in fp16, so hi*x + lo*x accumulated
in fp32 PSUM reproduces fp32-conv numerics to ~1e-7 relative. Spikes
out as fp16 0/1 (exact); counts accumulate in fp16 (integers <= 10,
exact) and are upcast on the host.
"""
import os
import time
import numpy as np
from contextlib import ExitStack

import concourse.tile as tile
import concourse.mybir as mybir
from concourse import bacc, bass_utils

N_TOTAL, T, CIN, COUT, H, W = 32, 10, 128, 128, 32, 32
NCORES = 8
NPC = N_TOTAL // NCORES          # samples per core
HP = WP = 34                     # padded image
HW = H * W                       # 1024
NCH = 512                        # matmul free-dim chunk (1 PSUM bank)
SCALE = np.float32(128.0)        # weight scale; x carries 1/128

_cache = {}


def _build_program(reps=1):
    """reps>1 repeats the whole per-core body (for benchmarking): same
    xpad input, distinct output regions per rep."""
    nsamp = NPC * reps
    nc = bacc.Bacc("TRN2", target_bir_lowering=False, debug=False)
    f16, f32 = mybir.dt.float16, mybir.dt.float32

    xpad_d = nc.dram_tensor("xpad", [NPC, T, CIN, HP * WP], f16, kind="ExternalInput").ap()
    whi_d = nc.dram_tensor("whi", [9, CIN, COUT], f16, kind="ExternalInput").ap()
    wlo_d = nc.dram_tensor("wlo", [9, CIN, COUT], f16, kind="ExternalInput").ap()
    bias_d = nc.dram_tensor("bias", [2, COUT], f16, kind="ExternalInput").ap()
    spk_d = nc.dram_tensor("spk", [nsamp, T, COUT, HW], f16, kind="ExternalOutput").ap()
    cnt_d = nc.dram_tensor("cnt", [nsamp, COUT, HW], f16, kind="ExternalOutput").ap()

    with tile.TileContext(nc) as tc, ExitStack() as ctx:
        const = ctx.enter_context(tc.tile_pool(name="const", bufs=1))
        xpool = ctx.enter_context(tc.tile_pool(name="xpool", bufs=4))
        spool = ctx.enter_context(tc.tile_pool(name="spool", bufs=6))
        cpool = ctx.enter_context(tc.tile_pool(name="cpool", bufs=1))
        mpool = ctx.enter_context(tc.tile_pool(name="mpool", bufs=1, space="PSUM"))

        bias_t = const.tile([2, COUT], f16)
        nc.sync.dma_start(bias_t[:], bias_d[:])
        ones_t = const.tile([2, NCH], f16)
        nc.vector.memset(ones_t[:], 1.0 / 128.0)
        # first sample's t=0 image right after bias, so it is in flight
        # while bias matmuls warm the PE
        xp_first = xpool.tile([CIN, HP * WP], f16, name="xp_first")
        nc.sync.dma_start(xp_first[:], xpad_d[0, 0])
        # k=0 hi-weights small DMA first, then the bulk
        whi_t = const.tile([CIN, 9 * COUT], f16)
        nc.sync.dma_start(whi_t[:, 0:COUT], whi_d[0])
        nc.sync.dma_start(whi_t[:, COUT:].rearrange("p (k c) -> p k c", k=8),
                          whi_d[1:9].rearrange("k p c -> p k c"))
        wlo_t = const.tile([CIN, 9 * COUT], f16)
        nc.sync.dma_start(wlo_t.rearrange("p (k c) -> p k c", k=9),
                          wlo_d.rearrange("k p c -> p k c"))

        for rep in range(reps):
            # all NPC membranes resident: 4 tiles x 2 banks = all 8 PSUM
            # banks. t-outer / n-inner interleave means each sample's DVE
            # threshold work hides behind the other samples' matmuls.
            mems = [mpool.tile([COUT, 2 * NCH], f32, tag=f"mem{n}",
                               name=f"mem{n}_{rep}") for n in range(NPC)]
            cnts = [cpool.tile([COUT, HW], f16, tag=f"cnt{n}",
                               name=f"cnt{n}_{rep}") for n in range(NPC)]

            # membrane init: mem = b_fold (hi+lo), via K=1 matmuls so the
            # PSUM has_written bits are set by the PE (start=True).
            for n in range(NPC):
                for c in range(2):
                    m = mems[n][:, NCH * c:NCH * (c + 1)]
                    nc.tensor.matmul(m, bias_t[:], ones_t[:],
                                     start=True, stop=False)

            for t in range(T):
                last_t = t == T - 1
                for n in range(NPC):
                    mem = mems[n]
                    if rep == 0 and t == 0 and n == 0:
                        xp_t = xp_first
                    else:
                        xp_t = xpool.tile([CIN, HP * WP], f16)
                        nc.sync.dma_start(xp_t[:], xpad_d[n, t])
                    xv = xp_t.rearrange("p (h w) -> p h w", w=WP)

                    for p_i, w_t in enumerate((whi_t, wlo_t)):
                        for k in range(9):
                            ky, kx = divmod(k, 3)
                            lhsT = w_t[:, k * COUT:(k + 1) * COUT]
                            for c in range(2):
                                nc.tensor.matmul(
                                    mem[:, NCH * c:NCH * (c + 1)], lhsT,
                                    xv[:, 16 * c + ky:16 * c + ky + 16, kx:kx + 32],
                                    start=False,
                                    stop=(last_t and p_i == 1 and k == 8))

                    spk_t = spool.tile([COUT, HW], f16)
                    nc.vector.tensor_scalar(spk_t[:], mem[:], 1.0, None,
                                            mybir.AluOpType.is_gt)
                    if not last_t:
                        nc.vector.tensor_tensor(mem[:], mem[:], spk_t[:],
                                                mybir.AluOpType.subtract)
                        if t == 0:
                            nc.gpsimd.tensor_copy(cnts[n][:], spk_t[:])
                        else:
                            nc.gpsimd.tensor_tensor(cnts[n][:], cnts[n][:], spk_t[:],
                                                    mybir.AluOpType.add)
                    nc.sync.dma_start(spk_d[rep * NPC + n, t], spk_t[:])

            for n in range(NPC):
                nc.sync.dma_start(cnt_d[rep * NPC + n], cnts[n][:])

    nc.compile()
    return nc


def _get_program():
    if "nc" not in _cache:
        _cache["nc"] = _build_program()
    return _cache["nc"]


def kernel(input_feature_st, input_features_sc, conv_w, conv_b,
           bn_gamma, bn_beta, bn_mean, bn_var):
    st = np.asarray(input_feature_st, dtype=np.float32)
    conv_w = np.asarray(conv_w, dtype=np.float32)
    conv_b = np.asarray(conv_b, dtype=np.float32)
    bn_gamma = np.asarray(bn_gamma, dtype=np.float32)
    bn_beta = np.asarray(bn_beta, dtype=np.float32)
    bn_mean = np.asarray(bn_mean, dtype=np.float32)
    bn_var = np.asarray(bn_var, dtype=np.float32)

    nc = _get_program()

    ratio = bn_gamma / np.sqrt(bn_var)
    w_fold = conv_w * ratio[:, None, None, None]          # [co, ci, kh, kw]
    b_fold = (conv_b - bn_mean) * ratio + bn_beta         # [co]

    # [9, ci, co] scaled weight splits
    w9 = np.ascontiguousarray(w_fold.transpose(2, 3, 1, 0).reshape(9, CIN, COUT)) * SCALE
    whi = w9.astype(np.float16)
    wlo = (w9 - whi.astype(np.float32)).astype(np.float16)
    bs = b_fold * SCALE
    bhi = bs.astype(np.float16)
    blo = (bs - bhi.astype(np.float32)).astype(np.float16)
    bias = np.stack([bhi, blo]).reshape(2, COUT)

    # host-side zero-pad to 34x34, value 1/128 where spiking (exact in fp16:
    # bit pattern 0x2000). Integer path is ~2x faster than a float cast.
    xpad_u = np.zeros((N_TOTAL, T, CIN, HP, WP), np.uint16)
    xpad_u[:, :, :, 1:H + 1, 1:W + 1] = st.astype(np.uint8) * np.uint16(0x2000)
    xpad = xpad_u.view(np.float16).reshape(N_TOTAL, T, CIN, HP * WP)

    in_maps = [{"xpad": xpad[c * NPC:(c + 1) * NPC],
                "whi": whi, "wlo": wlo, "bias": bias} for c in range(NCORES)]
    # NTFF tracing is not available under this axon build; force it off so a
    # stray BASS_TRACE env var can't break execution.
    os.environ.setdefault("BASS_NEVER_TRACE", "1")
    res = None
    for attempt in range(3):
        try:
            res = bass_utils.run_bass_kernel_spmd(nc, in_maps, list(range(NCORES)))
            break
        except Exception:
            # transient NRT/device hiccups recover on retry
            if attempt == 2:
                raise
            time.sleep(2.0)
    _cache["last_result"] = res

    out_st = np.empty((N_TOTAL, T, COUT, H, W), np.float32)
    out_sc = np.empty((N_TOTAL, COUT, H, W), np.float32)
    for c in range(NCORES):
        r = res.results[c]
        out_st[c * NPC:(c + 1) * NPC] = (
            r["spk"].reshape(NPC, T, COUT, H, W).astype(np.float32))
        out_sc[c * NPC:(c + 1) * NPC] = (
            r["cnt"].reshape(NPC, COUT, H, W).astype(np.float32))
    # device accumulates t=0..8 only; t=9's spikes are added here (exact ints)
    out_sc += out_st[:, T - 1]
    return out_st, out_sc
